# revision 2
# baseline (speedup 1.0000x reference)
"""Trainium2 kernel for nn_HATGNN: hierarchical label<-patch kNN aggregation.

Strategy (v2): the dominant work (832x100000 squared-euclidean cdist + top-9)
runs on 8 NeuronCores, patch-sharded (12500 rows/core, padded to 12800).

Per core, per 128-label chunk:
  PE:  z = (2L)@P_shard.T (bf16, f32 PSUM accum) - |p|^2 (fp16 via K=1 matmul)
       into [128, 2048]-column PSUM groups (4 banks), 512 cols per matmul.
  DVE: grouped tensor_reduce(max) over 16-wide patch tiles straight from PSUM
       -> [128, 800] f32 tile-maxima in SBUF, then one MAX8 + FIND_INDEX8
       -> top-8 tile indices per label.

Host merges 8 cores x 8 tiles x 16 patches = <=1024 candidates per label and
re-ranks them EXACTLY in f32 (so device numerics only have to rank tiles,
not patches).  Union safety: the core-local top-8 patches always lie in <=8
distinct tiles, so the union contains the global top-9 unless >=9 of them
fall in one core (p ~ 6e-8/label).  Validated offline: 0/832 labels miss,
rel err 2.7e-7.

The tiny 3-level MLP/LayerNorm pipeline (<=832 rows) runs in numpy.
"""
import numpy as np
import ml_dtypes

import concourse.bacc as bacc
import concourse.mybir as mybir
from concourse.tile import TileContext
from concourse.bass_utils import run_bass_kernel_spmd

NCORES = 8
NPER = 12500          # patches per core
NPAD = 12800          # padded (800 x 16)
RT = 16               # reduce-tile width (candidate granularity)
NT = NPAD // RT       # 800 tile maxima per label
MM = 512              # matmul moving width (1 PSUM bank of f32)
GROUPS = (4, 4, 4, 4, 3, 3, 3)   # matmul tiles per PSUM group (25 total)
S = 832               # total labels (64 mood + 256 genre + 512 sub)
SL = 896              # padded to 7 x 128
NCHUNK = SL // 128    # 7 label chunks
C = 256
EPS = 1e-5
PAD_NPSQ = -60000.0   # fp16-representable sentinel for padded patch columns

_CACHE = {}
LAST_RESULT = None    # BassKernelResults of the most recent device run


def _build_nc():
    bf16 = mybir.dt.bfloat16
    f16 = mybir.dt.float16
    f32 = mybir.dt.float32
    u32 = mybir.dt.uint32
    nc = bacc.Bacc()
    ptT = nc.dram_tensor("ptT", [2, 128, NPAD], bf16, kind="ExternalInput")
    lT = nc.dram_tensor("lT", [2, 128, SL], bf16, kind="ExternalInput")
    npsq = nc.dram_tensor("npsq", [1, NPAD], f16, kind="ExternalInput")
    onesw = nc.dram_tensor("onesw", [1, 128], f16, kind="ExternalInput")
    tidx = nc.dram_tensor("tidx", [SL, 8], u32, kind="ExternalOutput")

    with TileContext(nc) as tc:
        with tc.tile_pool(name="big", bufs=1) as bigp, \
             tc.tile_pool(name="work", bufs=2) as workp, \
             tc.tile_pool(name="ps", bufs=2, space="PSUM") as psp:
            pt0 = bigp.tile([128, NPAD], bf16, tag="pt0")
            pt1 = bigp.tile([128, NPAD], bf16, tag="pt1")
            lt0 = bigp.tile([128, SL], bf16, tag="lt0")
            lt1 = bigp.tile([128, SL], bf16, tag="lt1")
            npsq_t = bigp.tile([1, NPAD], f16, tag="npsq")
            ones_t = bigp.tile([1, 128], f16, tag="ones")

            nc.sync.dma_start(out=lt0[:], in_=lT[0])
            nc.sync.dma_start(out=lt1[:], in_=lT[1])
            nc.sync.dma_start(out=npsq_t[:], in_=npsq[:])
            nc.sync.dma_start(out=ones_t[:], in_=onesw[:])
            # patch halves in per-group column pieces so matmuls can start
            # as soon as their piece lands (subtile deps)
            col = 0
            for g_w in GROUPS:
                csl = slice(col, col + g_w * MM)
                nc.sync.dma_start(out=pt0[:, csl], in_=ptT[0][:, csl])
                nc.sync.dma_start(out=pt1[:, csl], in_=ptT[1][:, csl])
                col += g_w * MM

            for lc in range(NCHUNK):
                lsl = slice(lc * 128, (lc + 1) * 128)
                gm = workp.tile([128, NT], f32, tag="gm")
                col = 0
                t0 = 0
                for g_w in GROUPS:
                    ps = psp.tile([128, 128, RT], f32, tag="ps")
                    for j in range(g_w):
                        csl = slice(col + j * MM, col + (j + 1) * MM)
                        nc.tensor.matmul(ps[:, j * 32:(j + 1) * 32, :],
                                         lt0[:, lsl], pt0[:, csl],
                                         start=True, stop=False)
                    for j in range(g_w):
                        csl = slice(col + j * MM, col + (j + 1) * MM)
                        nc.tensor.matmul(ps[:, j * 32:(j + 1) * 32, :],
                                         lt1[:, lsl], pt1[:, csl],
                                         start=False, stop=False)
                    for j in range(g_w):
                        csl = slice(col + j * MM, col + (j + 1) * MM)
                        nc.tensor.matmul(ps[:, j * 32:(j + 1) * 32, :],
                                         ones_t[:], npsq_t[:, csl],
                                         start=False, stop=True)
                    nw = g_w * 32
                    nc.vector.tensor_reduce(out=gm[:, t0:t0 + nw],
                                            in_=ps[:, :nw, :],
                                            axis=mybir.AxisListType.X,
                                            op=mybir.AluOpType.max)
                    col += g_w * MM
                    t0 += nw
                mv = workp.tile([128, 8], f32, tag="mv")
                mi = workp.tile([128, 8], u32, tag="mi")
                nc.vector.max(out=mv[:], in_=gm[:])
                nc.vector.max_index(out=mi[:], in_max=mv[:], in_values=gm[:])
                nc.gpsimd.dma_start(out=tidx[lsl, :], in_=mi[:])
    nc.finalize()
    return nc


def _run_device(P, labels):
    """P: (100000, 256) f32, labels: (832, 256) f32.
    Returns tidx (8, 896, 8) int64: per-core top-8 16-wide tile ids/label."""
    global LAST_RESULT
    if "nc" not in _CACHE:
        _CACHE["nc"] = _build_nc()
    nc = _CACHE["nc"]

    L2 = np.zeros((SL, C), np.float32)
    L2[:S] = 2.0 * labels
    lT = np.ascontiguousarray(L2.T.astype(ml_dtypes.bfloat16)).reshape(2, 128, SL)

    in_maps = []
    for c in range(NCORES):
        sh = P[c * NPER:(c + 1) * NPER]     # (12500, 256)
        ptT = np.zeros((C, NPAD), ml_dtypes.bfloat16)
        ptT[:, :NPER] = sh.T.astype(ml_dtypes.bfloat16)
        npsq = np.full((1, NPAD), PAD_NPSQ, np.float16)
        npsq[0, :NPER] = -(sh.astype(np.float64) ** 2).sum(1).astype(np.float16)
        in_maps.append({
            "ptT": np.ascontiguousarray(ptT).reshape(2, 128, NPAD),
            "lT": lT,
            "npsq": npsq,
            "onesw": np.ones((1, 128), np.float16),
        })
    res = run_bass_kernel_spmd(nc, in_maps, core_ids=list(range(NCORES)))
    LAST_RESULT = res
    return np.stack([np.asarray(r["tidx"]) for r in res.results]).astype(np.int64)


def _merge_ctx(labels_sl, s0, s1, tidx, P, psq):
    """Exact re-rank of device tile candidates -> ctx = max(9 nbrs) - label."""
    n = s1 - s0
    t = tidx[:, s0:s1, :]                                    # (8, n, 8)
    loc = (t * RT)[..., None] + np.arange(RT)                # (8, n, 8, 16)
    valid = loc < NPER
    gid = loc + (np.arange(NCORES, dtype=np.int64) * NPER)[:, None, None, None]
    gid = np.where(valid, gid, 0)
    gid = gid.transpose(1, 0, 2, 3).reshape(n, -1)           # (n, 1024)
    valid = valid.transpose(1, 0, 2, 3).reshape(n, -1)

    # sort candidates by global id so a stable value-sort breaks ties
    # toward the smallest index (matches jax.lax.top_k)
    perm = np.argsort(gid, axis=1)
    gid = np.take_along_axis(gid, perm, 1)
    valid = np.take_along_axis(valid, perm, 1)

    out = np.empty((n, C), np.float32)
    B = 64
    for b0 in range(0, n, B):
        b1 = min(b0 + B, n)
        g = gid[b0:b1]                                       # (B, 1024)
        pv = P[g]                                            # (B, 1024, 256)
        d2 = psq[g] - 2.0 * np.einsum('bkc,bc->bk', pv, labels_sl[b0:b1],
                                      dtype=np.float32)
        d2[~valid[b0:b1]] = np.inf
        d2[:, 1:][g[:, 1:] == g[:, :-1]] = np.inf            # drop dup tiles
        sel = np.argsort(d2, axis=1, kind="stable")[:, :9]
        nbr = np.take_along_axis(pv, sel[..., None], 1)      # (B, 9, 256)
        out[b0:b1] = nbr.max(axis=1) - labels_sl[b0:b1]
    return out


def _label_topk_ctx(labels, tbl, k):
    """Small exact label<-label aggregation (matches reference ordering)."""
    d2 = ((labels * labels).sum(-1, keepdims=True)
          - 2.0 * labels @ tbl.T + (tbl * tbl).sum(-1)[None, :]).astype(np.float32)
    idx = np.argsort(d2, axis=1, kind="stable")[:, :k]
    nbrs = tbl[idx]
    return nbrs.max(axis=1) - labels


def _layer_norm(x, g, b):
    mu = x.mean(-1, keepdims=True)
    var = x.var(-1, keepdims=True)
    return (x - mu) / np.sqrt(var + EPS) * g + b


def kernel(patch_emb, mood_emb, genre_emb, sub_emb,
           Wm_w, Wm_b, Wg_w, Wg_b, Ws_w, Ws_b,
           lnm_g, lnm_b, lng_g, lng_b, lns_g, lns_b):
    P = np.ascontiguousarray(np.asarray(patch_emb, np.float32))
    mood_e = np.asarray(mood_emb, np.float32)
    genre_e = np.asarray(genre_emb, np.float32)
    sub_e = np.asarray(sub_emb, np.float32)
    labels = np.concatenate([mood_e, genre_e, sub_e], 0)

    tidx = _run_device(P, labels)
    psq = (P.astype(np.float64) ** 2).sum(1).astype(np.float32)

    ctx_m = _merge_ctx(mood_e, 0, 64, tidx, P, psq)
    mood = _layer_norm(mood_e + np.concatenate([mood_e, ctx_m], -1) @ np.asarray(Wm_w)
                       + np.asarray(Wm_b), np.asarray(lnm_g), np.asarray(lnm_b))

    ctx_gp = _merge_ctx(genre_e, 64, 320, tidx, P, psq)
    ctx_gm = _label_topk_ctx(genre_e, mood.astype(np.float32), 4)
    genre = _layer_norm(genre_e + np.concatenate([genre_e, ctx_gp, ctx_gm], -1)
                        @ np.asarray(Wg_w) + np.asarray(Wg_b),
                        np.asarray(lng_g), np.asarray(lng_b))

    ctx_sp = _merge_ctx(sub_e, 320, 832, tidx, P, psq)
    ctx_sm = _label_topk_ctx(sub_e, mood.astype(np.float32), 3)
    ctx_sg = _label_topk_ctx(sub_e, genre.astype(np.float32), 4)
    sub = _layer_norm(sub_e + np.concatenate([sub_e, ctx_sp, ctx_sm, ctx_sg], -1)
                      @ np.asarray(Ws_w) + np.asarray(Ws_b),
                      np.asarray(lns_g), np.asarray(lns_b))

    return np.concatenate([mood, genre, sub], 0).astype(np.float32)


# revision 6
# speedup vs baseline: 1.0004x; 1.0004x over previous
"""Trainium2 kernel for nn_HATGNN: hierarchical label<-patch kNN aggregation.

Strategy (v2): the dominant work (832x100000 squared-euclidean cdist + top-9)
runs on 8 NeuronCores, patch-sharded (12500 rows/core, padded to 12800).

Per core, per 128-label chunk:
  PE:  z = (2L)@P_shard.T (bf16, f32 PSUM accum) - |p|^2 (fp16 via K=1 matmul)
       into [128, 2048]-column PSUM groups (4 banks), 512 cols per matmul.
  DVE: grouped tensor_reduce(max) over 16-wide patch tiles straight from PSUM
       -> [128, 800] f32 tile-maxima in SBUF, then one MAX8 + FIND_INDEX8
       -> top-8 tile indices per label.

Host merges 8 cores x 8 tiles x 16 patches = <=1024 candidates per label and
re-ranks them EXACTLY in f32 (so device numerics only have to rank tiles,
not patches).  Union safety: the core-local top-8 patches always lie in <=8
distinct tiles, so the union contains the global top-9 unless >=9 of them
fall in one core (p ~ 6e-8/label).  Validated offline: 0/832 labels miss,
rel err 2.7e-7.

The tiny 3-level MLP/LayerNorm pipeline (<=832 rows) runs in numpy.
"""
import numpy as np
import ml_dtypes

import concourse.bacc as bacc
import concourse.mybir as mybir
from concourse.tile import TileContext
from concourse.bass_utils import run_bass_kernel_spmd

NCORES = 8
NPER = 12500          # patches per core
NPAD = 12800          # padded (800 x 16)
RT = 16               # reduce-tile width (candidate granularity)
NT = NPAD // RT       # 800 tile maxima per label
MM = 512              # matmul moving width (1 PSUM bank of f32)
GROUPS = (4, 4, 4, 4, 3, 3, 3)   # matmul tiles per PSUM group (25 total)
S = 832               # total labels (64 mood + 256 genre + 512 sub)
SL = 896              # padded to 7 x 128
NCHUNK = SL // 128    # 7 label chunks
C = 256
EPS = 1e-5
PAD_NPSQ = -60000.0   # fp16-representable sentinel for padded patch columns

_CACHE = {}
LAST_RESULT = None    # BassKernelResults of the most recent device run


def _dedupe_ldweights(nc):
    """Drop InstLdweights whose weights AP matches the previous load on the
    PE stream.  bass emits one LDWEIGHTS per matmul; with stationary-major
    matmul ordering most reloads are redundant (the PE array still holds the
    weights).  Only waitless/updateless LDWs are dropped, so semaphore
    structure is preserved."""
    for b in nc.m.functions[0].blocks:
        insts = b.instructions
        last_key = None
        drop = []
        for idx, i in enumerate(list(insts)):
            nm = type(i).__name__
            if nm == 'InstLdweights':
                w = i.ins[0]
                key = (str(w.memref), w.offset, tuple(map(tuple, w.ap)),
                       str(w.dtype), i.is_transpose, str(i.perf_mode))
                si = i.sync_info
                clean = si is None or (not si.on_wait and not si.on_update)
                if key == last_key and clean:
                    drop.append(idx)
                else:
                    last_key = key
            elif nm in ('InstMatmult', 'InstEventSemaphore'):
                pass
            elif str(i.engine) == 'EngineType.PE':
                last_key = None
        for idx in reversed(drop):
            del insts[idx]


def _build_nc():
    bf16 = mybir.dt.bfloat16
    f16 = mybir.dt.float16
    f32 = mybir.dt.float32
    u32 = mybir.dt.uint32
    nc = bacc.Bacc()
    ptT = nc.dram_tensor("ptT", [2, 128, NPAD], bf16, kind="ExternalInput")
    lT = nc.dram_tensor("lT", [2, 128, SL], bf16, kind="ExternalInput")
    npsq = nc.dram_tensor("npsq", [1, NPAD], f16, kind="ExternalInput")
    onesw = nc.dram_tensor("onesw", [1, 128], f16, kind="ExternalInput")
    tidx = nc.dram_tensor("tidx", [SL, 8], u32, kind="ExternalOutput")

    with TileContext(nc) as tc:
        with tc.tile_pool(name="big", bufs=1) as bigp, \
             tc.tile_pool(name="work", bufs=2) as workp, \
             tc.tile_pool(name="ps", bufs=2, space="PSUM") as psp:
            pt0 = bigp.tile([128, NPAD], bf16, tag="pt0")
            pt1 = bigp.tile([128, NPAD], bf16, tag="pt1")
            lt0 = bigp.tile([128, SL], bf16, tag="lt0")
            lt1 = bigp.tile([128, SL], bf16, tag="lt1")
            npsq_t = bigp.tile([1, NPAD], f16, tag="npsq")
            ones_t = bigp.tile([1, 128], f16, tag="ones")

            # first-needed data first; the two halves ride separate queues
            # (sync vs scalar) so they transfer in parallel.  patch halves
            # arrive in per-group column pieces so matmuls can start as soon
            # as their piece lands (subtile deps).
            nc.sync.dma_start(out=lt0[:], in_=lT[0])
            nc.sync.dma_start(out=pt0[:, 0:GROUPS[0] * MM],
                              in_=ptT[0][:, 0:GROUPS[0] * MM])
            nc.scalar.dma_start(out=lt1[:], in_=lT[1])
            nc.scalar.dma_start(out=pt1[:, 0:GROUPS[0] * MM],
                                in_=ptT[1][:, 0:GROUPS[0] * MM])
            nc.gpsimd.dma_start(out=npsq_t[:], in_=npsq[:])
            nc.gpsimd.dma_start(out=ones_t[:], in_=onesw[:])
            col = GROUPS[0] * MM
            for g_w in GROUPS[1:]:
                csl = slice(col, col + g_w * MM)
                nc.sync.dma_start(out=pt0[:, csl], in_=ptT[0][:, csl])
                nc.scalar.dma_start(out=pt1[:, csl], in_=ptT[1][:, csl])
                col += g_w * MM

            for lc in range(NCHUNK):
                lsl = slice(lc * 128, (lc + 1) * 128)
                gm = workp.tile([128, NT], f32, tag="gm")
                col = 0
                t0 = 0
                for g_w in GROUPS:
                    ps = psp.tile([128, 128, RT], f32, tag="ps")
                    for j in range(g_w):
                        csl = slice(col + j * MM, col + (j + 1) * MM)
                        nc.tensor.matmul(ps[:, j * 32:(j + 1) * 32, :],
                                         lt0[:, lsl], pt0[:, csl],
                                         start=True, stop=False)
                    for j in range(g_w):
                        csl = slice(col + j * MM, col + (j + 1) * MM)
                        nc.tensor.matmul(ps[:, j * 32:(j + 1) * 32, :],
                                         lt1[:, lsl], pt1[:, csl],
                                         start=False, stop=False)
                    for j in range(g_w):
                        csl = slice(col + j * MM, col + (j + 1) * MM)
                        nc.tensor.matmul(ps[:, j * 32:(j + 1) * 32, :],
                                         ones_t[:], npsq_t[:, csl],
                                         start=False, stop=True)
                    nw = g_w * 32
                    nc.vector.tensor_reduce(out=gm[:, t0:t0 + nw],
                                            in_=ps[:, :nw, :],
                                            axis=mybir.AxisListType.X,
                                            op=mybir.AluOpType.max)
                    col += g_w * MM
                    t0 += nw
                mv = workp.tile([128, 8], f32, tag="mv")
                mi = workp.tile([128, 8], u32, tag="mi")
                nc.vector.max(out=mv[:], in_=gm[:])
                nc.vector.max_index(out=mi[:], in_max=mv[:], in_values=gm[:])
                nc.gpsimd.dma_start(out=tidx[lsl, :], in_=mi[:])
    nc.finalize()
    _dedupe_ldweights(nc)
    return nc


def _run_device(P, labels):
    """P: (100000, 256) f32, labels: (832, 256) f32.
    Returns tidx (8, 896, 8) int64: per-core top-8 16-wide tile ids/label."""
    global LAST_RESULT
    if "nc" not in _CACHE:
        _CACHE["nc"] = _build_nc()
    nc = _CACHE["nc"]

    L2 = np.zeros((SL, C), np.float32)
    L2[:S] = 2.0 * labels
    lT = np.ascontiguousarray(L2.T.astype(ml_dtypes.bfloat16)).reshape(2, 128, SL)

    in_maps = []
    for c in range(NCORES):
        sh = P[c * NPER:(c + 1) * NPER]     # (12500, 256)
        ptT = np.zeros((C, NPAD), ml_dtypes.bfloat16)
        ptT[:, :NPER] = sh.T.astype(ml_dtypes.bfloat16)
        npsq = np.full((1, NPAD), PAD_NPSQ, np.float16)
        npsq[0, :NPER] = -(sh.astype(np.float64) ** 2).sum(1).astype(np.float16)
        in_maps.append({
            "ptT": np.ascontiguousarray(ptT).reshape(2, 128, NPAD),
            "lT": lT,
            "npsq": npsq,
            "onesw": np.ones((1, 128), np.float16),
        })
    res = run_bass_kernel_spmd(nc, in_maps, core_ids=list(range(NCORES)))
    LAST_RESULT = res
    return np.stack([np.asarray(r["tidx"]) for r in res.results]).astype(np.int64)


def _merge_ctx(labels_sl, s0, s1, tidx, P, psq):
    """Exact re-rank of device tile candidates -> ctx = max(9 nbrs) - label."""
    n = s1 - s0
    t = tidx[:, s0:s1, :]                                    # (8, n, 8)
    loc = (t * RT)[..., None] + np.arange(RT)                # (8, n, 8, 16)
    valid = loc < NPER
    gid = loc + (np.arange(NCORES, dtype=np.int64) * NPER)[:, None, None, None]
    gid = np.where(valid, gid, 0)
    gid = gid.transpose(1, 0, 2, 3).reshape(n, -1)           # (n, 1024)
    valid = valid.transpose(1, 0, 2, 3).reshape(n, -1)

    # sort candidates by global id so a stable value-sort breaks ties
    # toward the smallest index (matches jax.lax.top_k)
    perm = np.argsort(gid, axis=1)
    gid = np.take_along_axis(gid, perm, 1)
    valid = np.take_along_axis(valid, perm, 1)

    out = np.empty((n, C), np.float32)
    B = 64
    for b0 in range(0, n, B):
        b1 = min(b0 + B, n)
        g = gid[b0:b1]                                       # (B, 1024)
        pv = P[g]                                            # (B, 1024, 256)
        d2 = psq[g] - 2.0 * np.einsum('bkc,bc->bk', pv, labels_sl[b0:b1],
                                      dtype=np.float32)
        d2[~valid[b0:b1]] = np.inf
        d2[:, 1:][g[:, 1:] == g[:, :-1]] = np.inf            # drop dup tiles
        sel = np.argsort(d2, axis=1, kind="stable")[:, :9]
        nbr = np.take_along_axis(pv, sel[..., None], 1)      # (B, 9, 256)
        out[b0:b1] = nbr.max(axis=1) - labels_sl[b0:b1]
    return out


def _label_topk_ctx(labels, tbl, k):
    """Small exact label<-label aggregation (matches reference ordering)."""
    d2 = ((labels * labels).sum(-1, keepdims=True)
          - 2.0 * labels @ tbl.T + (tbl * tbl).sum(-1)[None, :]).astype(np.float32)
    idx = np.argsort(d2, axis=1, kind="stable")[:, :k]
    nbrs = tbl[idx]
    return nbrs.max(axis=1) - labels


def _layer_norm(x, g, b):
    mu = x.mean(-1, keepdims=True)
    var = x.var(-1, keepdims=True)
    return (x - mu) / np.sqrt(var + EPS) * g + b


def kernel(patch_emb, mood_emb, genre_emb, sub_emb,
           Wm_w, Wm_b, Wg_w, Wg_b, Ws_w, Ws_b,
           lnm_g, lnm_b, lng_g, lng_b, lns_g, lns_b):
    P = np.ascontiguousarray(np.asarray(patch_emb, np.float32))
    mood_e = np.asarray(mood_emb, np.float32)
    genre_e = np.asarray(genre_emb, np.float32)
    sub_e = np.asarray(sub_emb, np.float32)
    labels = np.concatenate([mood_e, genre_e, sub_e], 0)

    tidx = _run_device(P, labels)
    psq = (P.astype(np.float64) ** 2).sum(1).astype(np.float32)

    ctx_m = _merge_ctx(mood_e, 0, 64, tidx, P, psq)
    mood = _layer_norm(mood_e + np.concatenate([mood_e, ctx_m], -1) @ np.asarray(Wm_w)
                       + np.asarray(Wm_b), np.asarray(lnm_g), np.asarray(lnm_b))

    ctx_gp = _merge_ctx(genre_e, 64, 320, tidx, P, psq)
    ctx_gm = _label_topk_ctx(genre_e, mood.astype(np.float32), 4)
    genre = _layer_norm(genre_e + np.concatenate([genre_e, ctx_gp, ctx_gm], -1)
                        @ np.asarray(Wg_w) + np.asarray(Wg_b),
                        np.asarray(lng_g), np.asarray(lng_b))

    ctx_sp = _merge_ctx(sub_e, 320, 832, tidx, P, psq)
    ctx_sm = _label_topk_ctx(sub_e, mood.astype(np.float32), 3)
    ctx_sg = _label_topk_ctx(sub_e, genre.astype(np.float32), 4)
    sub = _layer_norm(sub_e + np.concatenate([sub_e, ctx_sp, ctx_sm, ctx_sg], -1)
                      @ np.asarray(Ws_w) + np.asarray(Ws_b),
                      np.asarray(lns_g), np.asarray(lns_b))

    return np.concatenate([mood, genre, sub], 0).astype(np.float32)
